# revision 32
# baseline (speedup 1.0000x reference)
"""Trainium2 Bass kernel for nn_DuhamelLayer (8-channel long-FIR conv1d).

Math: out[b,o,t] = sum_k irf[o,k] * x[b, t+k-pad]  (cross-correlation,
'SAME' padding, pad = MAXK//2).  The conv is recast as a chain of
PSUM-accumulating 128x128 Toeplitz-block matmuls on the TensorEngine:

  t = 128*a + p,  k = 128*c + (u - p)
  out[p, a] = sum_c sum_u M_c[u, p] * X[u, a + c]
  M_c[u, p] = w[128*c + u - p]           (dense Toeplitz block)
  X[u, m]   = xpad[128*m + u]            (partition-fast input layout)

Only blocks with any nonzero tap are emitted (66 of 8*16 possible).
Sharding: data-parallel over batch, 2 batches per core x 8 cores.
Host side does layout transforms only (pad/transpose/weight prep).
"""

import numpy as np

# ---- static config (mirrors the nn.Module) ----
OMEGAS = [5.0, 7.0, 9.0, 12.0, 16.0, 22.0, 30.0, 40.0]
XI = 0.05
DT = 0.01
UJ_U1 = 0.01

_decay = (1.0 / (2.0 * np.pi * XI)) * np.log(1.0 / UJ_U1)
VALID_W = [int(2.0 * np.pi / w / np.sqrt(1.0 - XI**2) * _decay / DT) for w in OMEGAS]
KER = [2 * a - 1 for a in VALID_W]
MAXK = max(KER)          # 3687
OUT_CH = len(OMEGAS)     # 8
PAD = MAXK // 2          # 1843

B = 16                   # batch
T = 65536                # sequence length
NCORES = 8
BPC = B // NCORES        # 2 batches per core
A = T // 128             # 512 output columns per (b, o) tile

# matmul dtype: "float32" (exact, 4 cyc/row) or "float32r" (1 cyc/row @ N>=256)
MM_DTYPE = "float32r"
MODE = "tile"            # "tile" (TileContext) or "raw" (manual semaphores)
TRACE = False            # test.py flips this for profiling
TRACE_KWARGS = {}
LAST_RESULTS = None

_NC_CACHE = {}


def _build_wbank(log_omegas):
    """float32 numpy mirror of the reference's _build_irfs -> [OUT_CH, MAXK]."""
    lo = np.asarray(log_omegas, dtype=np.float32)
    omegas = np.clip(np.exp(lo), 0.01, 1000.0).astype(np.float32)
    sq = np.float32(np.sqrt(np.float32(1.0 - XI**2)))
    rows = []
    for i in range(OUT_CH):
        W, K = VALID_W[i], KER[i]
        tt = (np.arange(W, dtype=np.float32) * np.float32(DT)).astype(np.float32)
        omegaD = np.float32(omegas[i] * sq)
        irf = (
            (np.float32(1.0) / omegaD)
            * np.exp((-np.float32(XI) * omegas[i]) * tt)
            * np.sin(omegaD * tt)
        ).astype(np.float32)
        w = np.concatenate([irf[::-1], np.zeros((K // 2,), np.float32)])
        addpad = MAXK - K
        w = np.pad(w, (addpad // 2, addpad // 2))
        rows.append(w)
    return np.stack(rows)


def _plan_blocks(wbank):
    """Per channel, the list of Toeplitz block indices c with any nonzero tap."""
    ncand = (MAXK + 127) // 128 + 1
    blocks = []
    for o in range(OUT_CH):
        nz = np.nonzero(wbank[o])[0]
        kmin, kmax = int(nz.min()), int(nz.max())
        cs = [
            c
            for c in range(ncand)
            if 128 * c + 127 >= kmin and 128 * c - 127 <= kmax
        ]
        blocks.append(cs)
    return blocks


def _build_weight_mats(wbank, blocks):
    """Per channel: [128, nblk*128] with column block i = M_{c_i}[u, p]."""
    u = np.arange(128)[:, None]
    p = np.arange(128)[None, :]
    mats = []
    for o in range(OUT_CH):
        cols = []
        for c in blocks[o]:
            idx = 128 * c + u - p
            valid = (idx >= 0) & (idx < MAXK)
            cols.append(
                np.where(valid, wbank[o][np.clip(idx, 0, MAXK - 1)], np.float32(0.0))
            )
        mats.append(np.ascontiguousarray(np.concatenate(cols, axis=1), np.float32))
    return mats


def _build_nc(blocks, xcols, mm_dtype):
    import concourse.bacc as bacc
    import concourse.mybir as mybir
    from concourse.tile import TileContext

    mm_dt = getattr(mybir.dt, mm_dtype)
    f32 = mybir.dt.float32

    nc = bacc.Bacc("TRN2", target_bir_lowering=False, debug=False)
    # x for both batches interleaved per partition: [u, b*xcols + m] so one
    # DMA moves 4216 B contiguous per partition (2x the descriptor size of
    # per-batch loads).
    xt_d = nc.dram_tensor("xt", [128, BPC * xcols], mm_dt, kind="ExternalInput")
    w_d = [
        nc.dram_tensor(f"wt{o}", [128, len(blocks[o]) * 128], mm_dt, kind="ExternalInput")
        for o in range(OUT_CH)
    ]
    y_d = nc.dram_tensor("y", [BPC, OUT_CH, 128, A], f32, kind="ExternalOutput")

    with TileContext(nc) as tc:
        with (
            tc.tile_pool(name="w", bufs=1) as wpool,
            tc.tile_pool(name="x", bufs=1) as xpool,
            tc.tile_pool(name="warm", bufs=1) as warmpool,
            tc.tile_pool(name="ps", bufs=3, space="PSUM") as pspool,
            tc.tile_pool(name="wps", bufs=1, space="PSUM") as wpspool,
            tc.tile_pool(name="o", bufs=4) as opool,
        ):
            # PE warm-up: dependency-free matmuls bridge the input-DMA wait
            # so HAM un-throttles (1.2 -> 2.4 GHz) before the real stream.
            # The result bank is never read.
            warm = warmpool.tile([128, 256], f32, name="warm")
            nc.vector.memset(warm[:], 0.0)
            warm_ps = wpspool.tile([128, 128], f32, name="warmps")
            for _ in range(8):
                nc.tensor.matmul(
                    warm_ps[:], warm[:, :128], warm[:, 128:], start=True, stop=True
                )
            # x first so it isn't queued behind 4.2 MB of weights on the
            # sync HWDGE ring.
            xt = xpool.tile([128, BPC * xcols], mm_dt, name="xt")
            nc.sync.dma_start(xt[:], xt_d[:])
            xtiles = [xt[:, b * xcols : (b + 1) * xcols] for b in range(BPC)]
            # smallest channels first (first matmul only waits on x plus a
            # 0.25 MB weight tile), but rotate the largest channel to
            # second-to-last: a small channel's matmuls then overlap the big
            # channel's 512 KB output DMA instead of it draining after the
            # stream ends.
            order = sorted(range(OUT_CH), key=lambda o: len(blocks[o]))
            order = order[:1] + order[2:] + order[1:2]
            wtiles = [None] * OUT_CH
            for o in order:
                wt = wpool.tile(
                    [128, len(blocks[o]) * 128], mm_dt, tag=f"w{o}", name=f"w{o}"
                )
                nc.sync.dma_start(wt[:], w_d[o][:])
                wtiles[o] = wt
            # (o, c) outer / b inner: each Toeplitz weight block loads into
            # the PE array once and streams both batches' columns.
            for o in order:
                cs = blocks[o]
                pss = [
                    pspool.tile([128, A], f32, tag=f"ps{b}", name=f"ps{o}_{b}")
                    for b in range(BPC)
                ]
                for i, c in enumerate(cs):
                    for b in range(BPC):
                        nc.tensor.matmul(
                            pss[b][:],
                            wtiles[o][:, i * 128 : (i + 1) * 128],
                            xtiles[b][:, c : c + A],
                            start=(i == 0),
                            stop=(i == len(cs) - 1),
                        )
                for b in range(BPC):
                    ot = opool.tile([128, A], f32, tag=f"ot{b}", name=f"ot{o}_{b}")
                    nc.vector.tensor_copy(ot[:], pss[b][:])
                    nc.sync.dma_start(y_d[b, o], ot[:])
    nc.compile()
    return nc


def _build_nc_raw(blocks, xcols, mm_dtype):
    """Manual-semaphore bacc version: no Tile teardown barriers (~8 us)."""
    import concourse.bacc as bacc
    import concourse.mybir as mybir

    mm_dt = getattr(mybir.dt, mm_dtype)
    f32 = mybir.dt.float32

    nc = bacc.Bacc("TRN2", target_bir_lowering=False, debug=False)
    xt_d = nc.dram_tensor("xt", [128, BPC * xcols], mm_dt, kind="ExternalInput")
    w_d = [
        nc.dram_tensor(f"wt{o}", [128, len(blocks[o]) * 128], mm_dt, kind="ExternalInput")
        for o in range(OUT_CH)
    ]
    y_d = nc.dram_tensor("y", [BPC, OUT_CH, 128, A], f32, kind="ExternalOutput")

    order = sorted(range(OUT_CH), key=lambda o: len(blocks[o]))
    NWARM = 8
    NSLOT = 4  # psum slots; slot s holds banks for (b0, b1) of channel k=s mod 4

    from contextlib import ExitStack

    with ExitStack() as ctx:
        xt = ctx.enter_context(nc.sbuf_tensor("xts", [128, BPC * xcols], mm_dt))
        warm = ctx.enter_context(nc.sbuf_tensor("warms", [128, 256], f32))
        wts = [
            ctx.enter_context(
                nc.sbuf_tensor(f"wts{o}", [128, len(blocks[o]) * 128], mm_dt)
            )
            for o in range(OUT_CH)
        ]
        ots = [
            ctx.enter_context(nc.sbuf_tensor(f"ots{j}", [128, A], f32))
            for j in range(4)
        ]
        pss = [
            [
                ctx.enter_context(nc.psum_tensor(f"rps{s}_{b}", [128, A], f32))
                for b in range(BPC)
            ]
            for s in range(NSLOT)
        ]
        # one semaphore per DMA: the 16 SDMA engines complete their shares of
        # successive same-ring DMAs out of order, so cumulative thresholds on
        # a shared semaphore do NOT imply per-DMA completion.
        xs = ctx.enter_context(nc.semaphore("xs"))
        wsem = [ctx.enter_context(nc.semaphore(f"ws{o}")) for o in range(OUT_CH)]
        osem = [ctx.enter_context(nc.semaphore(f"os{i}")) for i in range(2 * OUT_CH)]
        mm_done = ctx.enter_context(nc.semaphore("mm_done"))
        copy_done = ctx.enter_context(nc.semaphore("copy_done"))
        block = ctx.enter_context(nc.Block())

        @block.sync
        def _(sync):
            sync.dma_start(xt[:], xt_d[:]).then_inc(xs, 16)
            for o in order:
                sync.dma_start(wts[o][:], w_d[o][:]).then_inc(wsem[o], 16)
            for i in range(2 * OUT_CH):
                k, b = divmod(i, 2)
                sync.wait_ge(copy_done, i + 1)
                sync.dma_start(y_d[b, order[k]], ots[i % 4][:]).then_inc(osem[i], 16)
            for i in range(2 * OUT_CH):
                sync.wait_ge(osem[i], 16)

        @block.tensor
        def _(tensor):
            # warm-up on uninitialized SBUF; pss[0][0] is cleared by the
            # first real matmul's start=True before any reader touches it.
            for _ in range(NWARM):
                tensor.matmul(
                    pss[0][0][:, :128], warm[:, :128], warm[:, 128:],
                    start=True, stop=True,
                )
            tensor.wait_ge(xs, 16)
            for k, o in enumerate(order):
                cs = blocks[o]
                tensor.wait_ge(wsem[o], 16)
                if k >= NSLOT:
                    # bank reuse: both copies of channel k-NSLOT drained
                    tensor.wait_ge(copy_done, 2 * (k - NSLOT) + 2)
                slot = pss[k % NSLOT]
                for i, c in enumerate(cs):
                    for b in range(BPC):
                        mm = tensor.matmul(
                            slot[b][:],
                            wts[o][:, i * 128 : (i + 1) * 128],
                            xt[:, b * xcols + c : b * xcols + c + A],
                            start=(i == 0),
                            stop=(i == len(cs) - 1),
                        )
                        if i == len(cs) - 1:
                            mm.then_inc(mm_done, 1)

        @block.vector
        def _(vector):
            for i in range(2 * OUT_CH):
                k, b = divmod(i, 2)
                vector.wait_ge(mm_done, i + 1)
                if i >= 4:
                    # out-slot reuse: DMA of copy i-4 complete
                    vector.wait_ge(osem[i - 4], 16)
                vector.tensor_copy(ots[i % 4][:], pss[k % NSLOT][b][:]).then_inc(
                    copy_done, 1
                )

    nc.compile()
    return nc


def kernel(inputs, log_omegas):
    global LAST_RESULTS
    from concourse.bass_utils import run_bass_kernel_spmd

    x = np.asarray(inputs, dtype=np.float32).reshape(B, T)
    wbank = _build_wbank(log_omegas)
    blocks = _plan_blocks(wbank)
    cmax = max(c for cs in blocks for c in cs)
    xcols = A + cmax
    assert xcols * 128 >= PAD + T, "input padding does not fit block reach"
    wmats = _build_weight_mats(wbank, blocks)

    # X[b][u, m] = xpad[b][128*m + u], xpad = [PAD zeros | x | tail zeros]
    xpad = np.zeros((B, xcols * 128), np.float32)
    xpad[:, PAD : PAD + T] = x
    xt_all = xpad.reshape(B, xcols, 128).transpose(0, 2, 1)  # [B, 128, xcols]
    # per core: both batches side by side in the free dim -> [128, BPC*xcols]
    xt_core = np.ascontiguousarray(
        xt_all.reshape(NCORES, BPC, 128, xcols).transpose(0, 2, 1, 3).reshape(
            NCORES, 128, BPC * xcols
        )
    )

    key = (tuple(tuple(cs) for cs in blocks), xcols, MM_DTYPE, MODE)
    if key not in _NC_CACHE:
        build = _build_nc_raw if MODE == "raw" else _build_nc
        _NC_CACHE[key] = build(blocks, xcols, MM_DTYPE)
    nc = _NC_CACHE[key]

    in_maps = []
    for i in range(NCORES):
        m = {"xt": xt_core[i]}
        for o in range(OUT_CH):
            m[f"wt{o}"] = wmats[o]
        in_maps.append(m)

    res = run_bass_kernel_spmd(
        nc, in_maps, list(range(NCORES)), trace=TRACE, **TRACE_KWARGS
    )
    LAST_RESULTS = res

    # y_dev[b_loc, o, p, a] = y[b, o, 128*a + p]
    y = np.empty((B, OUT_CH, T), np.float32)
    for i in range(NCORES):
        arr = res.results[i]["y"]
        for b in range(BPC):
            y[i * BPC + b] = arr[b].transpose(0, 2, 1).reshape(OUT_CH, T)
    return y.reshape(B, OUT_CH, T)
